# revision 2
# baseline (speedup 1.0000x reference)
"""Single-head causal self-attention for Trainium2, data-parallel over batch.

Problem: x[B=8, T=2048, D=1024], Wq/Wk/Wv[1024, 1024] (fp32).
  q/k/v = x @ W*, scores = (q @ k^T)/sqrt(H) causal-masked, out = softmax @ v.

Sharding: one batch element per NeuronCore (8 cores). Each core runs an
identical Bass/Tile program on its own x[b].

Per-core dataflow (all matmul compute in bf16, accumulation fp32):
  1. xT[d, t]  <- DMA-transpose of x (bf16) ; W* cast to bf16.
  2. QT[h, t] = Wq.T @ x.T ; KT likewise ; V[t, h] = x @ Wv.
  3. Scores are computed TRANSPOSED: ST[tk, tq] = K @ Q^T, so
     PT = exp(ST/sqrt(H)) (causal-masked via affine_select) is directly the
     stationary operand for O[tq, h] = PT.T @ V — no transposes of the
     softmax weights or the output are ever needed.
  4. Row-sums r[tq] accumulate in PSUM via an extra N=1 matmul against a
     ones column; O is normalized by 1/r during the PSUM->SBUF copy.
"""

import numpy as np

P = 128
STRIP = 512  # free-dim strip for N=512 matmuls (one fp32 PSUM bank)


def build_nc(T=2048, D=1024, H=1024):
    import concourse.bacc as bacc
    import concourse.mybir as mybir
    import concourse.tile as tile

    F32 = mybir.dt.float32
    BF16 = mybir.dt.bfloat16
    EXP = mybir.ActivationFunctionType.Exp

    nT, nD, nH = T // P, D // P, H // P
    nTS, nHS = T // STRIP, H // STRIP
    tps = STRIP // P  # t-tiles per strip
    scale = 1.0 / float(np.sqrt(H))

    nc = bacc.Bacc("TRN2", target_bir_lowering=False, debug=False)
    x = nc.dram_tensor("x", (T, D), F32, kind="ExternalInput").ap()
    Wq = nc.dram_tensor("Wq", (D, H), F32, kind="ExternalInput").ap()
    Wk = nc.dram_tensor("Wk", (D, H), F32, kind="ExternalInput").ap()
    Wv = nc.dram_tensor("Wv", (D, H), F32, kind="ExternalInput").ap()
    out = nc.dram_tensor("out", (T, H), F32, kind="ExternalOutput").ap()

    with tile.TileContext(nc) as tc:
        with tc.tile_pool(name="persist", bufs=1) as persist:
            ones_col = persist.tile([P, 1], BF16, name="ones_col")
            nc.vector.memset(ones_col, 1.0)
            QT = [persist.tile([P, T], BF16, name=f"qt{h}") for h in range(nH)]
            KT = [persist.tile([P, T], BF16, name=f"kt{h}") for h in range(nH)]
            V = [persist.tile([P, H], BF16, name=f"v{t}") for t in range(nT)]

            with tc.tile_pool(name="proj", bufs=1) as proj, \
                 tc.tile_pool(name="stage", bufs=3) as stage, \
                 tc.tile_pool(name="ppsum", bufs=4, space="PSUM") as ppsum:
                xT = [proj.tile([P, T], BF16, name=f"xt{d}") for d in range(nD)]
                Wqb = [proj.tile([P, H], BF16, name=f"wqb{d}") for d in range(nD)]
                Wkb = [proj.tile([P, H], BF16, name=f"wkb{d}") for d in range(nD)]
                Wvb = [proj.tile([P, H], BF16, name=f"wvb{d}") for d in range(nD)]

                # Weight loads: SWDGE casting DMA fp32 -> bf16.
                for d in range(nD):
                    nc.gpsimd.dma_start(Wqb[d], Wq[d * P:(d + 1) * P, :])
                    nc.gpsimd.dma_start(Wkb[d], Wk[d * P:(d + 1) * P, :])
                    nc.gpsimd.dma_start(Wvb[d], Wv[d * P:(d + 1) * P, :])

                # x: casting load then 128x128 SBUF->SBUF DMA transposes.
                for t in range(nT):
                    xb = stage.tile([P, D], BF16, name=f"xb{t}", tag="xb")
                    nc.gpsimd.dma_start(xb, x[t * P:(t + 1) * P, :])
                    for d in range(nD):
                        nc.sync.dma_start(
                            xT[d][:, t * P:(t + 1) * P],
                            xb[:, d * P:(d + 1) * P],
                            transpose=True,
                        )

                # Projections. QT/KT: [h, t] = W*.T @ x.T ; V: [t, h] = x @ Wv.
                ncp = 0
                for Wb, OUT in ((Wqb, QT), (Wkb, KT)):
                    for h in range(nH):
                        for ts in range(nTS):
                            ps = ppsum.tile([P, STRIP], F32,
                                            name=f"ps{ncp}", tag="ps")
                            for d in range(nD):
                                nc.tensor.matmul(
                                    ps,
                                    Wb[d][:, h * P:(h + 1) * P],
                                    xT[d][:, ts * STRIP:(ts + 1) * STRIP],
                                    start=(d == 0), stop=(d == nD - 1),
                                )
                            dst = OUT[h][:, ts * STRIP:(ts + 1) * STRIP]
                            if ncp % 2 == 0:
                                nc.vector.tensor_copy(dst, ps)
                            else:
                                nc.scalar.copy(dst, ps)
                            ncp += 1
                for t in range(nT):
                    for hs in range(nHS):
                        ps = ppsum.tile([P, STRIP], F32, name=f"ps{ncp}", tag="ps")
                        for d in range(nD):
                            nc.tensor.matmul(
                                ps,
                                xT[d][:, t * P:(t + 1) * P],
                                Wvb[d][:, hs * STRIP:(hs + 1) * STRIP],
                                start=(d == 0), stop=(d == nD - 1),
                            )
                        if ncp % 2 == 0:
                            nc.vector.tensor_copy(
                                V[t][:, hs * STRIP:(hs + 1) * STRIP], ps)
                        else:
                            nc.scalar.copy(
                                V[t][:, hs * STRIP:(hs + 1) * STRIP], ps)
                        ncp += 1

            # Attention, strip by strip over tq.
            with tc.tile_pool(name="ptpool", bufs=2) as ptpool, \
                 tc.tile_pool(name="ostage", bufs=3) as ostage, \
                 tc.tile_pool(name="small", bufs=4) as small, \
                 tc.tile_pool(name="stpsum", bufs=2, space="PSUM") as stpsum, \
                 tc.tile_pool(name="opsum", bufs=2, space="PSUM") as opsum:
                for s in range(nTS):
                    q0 = s * STRIP
                    pts = []
                    for k in range((s + 1) * tps):  # tk tiles with any live tq
                        jq0 = max(0, k * P - q0)  # first unmasked col in strip
                        N = STRIP - jq0
                        st = stpsum.tile([P, STRIP], F32,
                                         name=f"st{s}_{k}", tag="st")
                        for h in range(nH):
                            nc.tensor.matmul(
                                st[:, :N],
                                KT[h][:, k * P:(k + 1) * P],
                                QT[h][:, q0 + jq0:q0 + STRIP],
                                start=(h == 0), stop=(h == nH - 1),
                            )
                        pt = ptpool.tile([P, STRIP], BF16,
                                         name=f"pt{s}_{k}", tag=f"pt{k}")
                        nc.scalar.activation(pt[:, jq0:STRIP], st[:, :N],
                                             EXP, scale=scale)
                        if k * P >= q0:
                            # Diagonal-crossing tile: zero where tk > tq.
                            # iota expr = j - p  (>=0 keeps exp, else 0).
                            nc.gpsimd.affine_select(
                                out=pt[:, jq0:STRIP], in_=pt[:, jq0:STRIP],
                                compare_op=mybir.AluOpType.is_ge,
                                fill=0.0, base=0, channel_multiplier=-1,
                                pattern=[[1, N]],
                            )
                        pts.append(pt)

                    for i in range(tps):
                        t = s * tps + i
                        ops = opsum.tile([P, H + 1], F32, name=f"o{t}", tag="o")
                        for k in range(t + 1):
                            lhsT = pts[k][:, i * P:(i + 1) * P]
                            first, last = (k == 0), (k == t)
                            for hs in range(nHS):
                                nc.tensor.matmul(
                                    ops[:, hs * STRIP:(hs + 1) * STRIP],
                                    lhsT,
                                    V[k][:, hs * STRIP:(hs + 1) * STRIP],
                                    start=first, stop=last,
                                )
                            nc.tensor.matmul(ops[:, H:H + 1], lhsT, ones_col,
                                             start=first, stop=last)
                        rinv = small.tile([P, 1], F32, name=f"rinv{t}",
                                          tag="rinv")
                        nc.vector.reciprocal(rinv, ops[:, H:H + 1])
                        osb = ostage.tile([P, H], F32, name=f"osb{t}", tag="osb")
                        nc.vector.tensor_scalar_mul(osb, ops[:, 0:H], rinv)
                        nc.sync.dma_start(out[t * P:(t + 1) * P, :], osb)

    nc.compile()
    return nc


def kernel(x, Wq, Wk, Wv):
    from concourse import bass_utils

    B, T, D = x.shape
    H = Wq.shape[1]
    nc = build_nc(T=T, D=D, H=H)
    in_maps = [
        {
            "x": np.ascontiguousarray(x[b], dtype=np.float32),
            "Wq": np.asarray(Wq, dtype=np.float32),
            "Wk": np.asarray(Wk, dtype=np.float32),
            "Wv": np.asarray(Wv, dtype=np.float32),
        }
        for b in range(B)
    ]
    res = bass_utils.run_bass_kernel_spmd(nc, in_maps, core_ids=list(range(B)))
    return np.stack([res.results[b]["out"] for b in range(B)], axis=0)


# revision 4
# speedup vs baseline: 1.4302x; 1.4302x over previous
"""Single-head causal self-attention for Trainium2, data-parallel over batch.

Problem: x[B=8, T=2048, D=1024], Wq/Wk/Wv[1024, 1024] (fp32).
  q/k/v = x @ W*, scores = (q @ k^T)/sqrt(H) causal-masked, out = softmax @ v.

Sharding: one batch element per NeuronCore (8 cores). Each core runs an
identical Bass/Tile program on its own x[b].

Per-core dataflow (all matmul compute in bf16, accumulation fp32):
  1. xT[d, t]  <- DMA-transpose of x (bf16) ; W* cast to bf16.
  2. QT[h, t] = Wq.T @ x.T ; KT likewise ; V[t, h] = x @ Wv.
  3. Scores are computed TRANSPOSED: ST[tk, tq] = K @ Q^T, so
     PT = exp(ST/sqrt(H)) (causal-masked via affine_select) is directly the
     stationary operand for O[tq, h] = PT.T @ V — no transposes of the
     softmax weights or the output are ever needed.
  4. Row-sums r[tq] accumulate in PSUM via an extra N=1 matmul against a
     ones column; O is normalized by 1/r during the PSUM->SBUF copy.
"""

import numpy as np

P = 128
STRIP = 512  # free-dim strip for N=512 matmuls (one fp32 PSUM bank)


def build_nc(T=2048, D=1024, H=1024):
    import concourse.bacc as bacc
    import concourse.mybir as mybir
    import concourse.tile as tile

    F32 = mybir.dt.float32
    BF16 = mybir.dt.bfloat16
    EXP = mybir.ActivationFunctionType.Exp

    nT, nD, nH = T // P, D // P, H // P
    nTS, nHS = T // STRIP, H // STRIP
    tps = STRIP // P  # t-tiles per strip
    scale = 1.0 / float(np.sqrt(H))

    nc = bacc.Bacc("TRN2", target_bir_lowering=False, debug=False)
    x = nc.dram_tensor("x", (T, D), F32, kind="ExternalInput").ap()
    Wq = nc.dram_tensor("Wq", (D, H), F32, kind="ExternalInput").ap()
    Wk = nc.dram_tensor("Wk", (D, H), F32, kind="ExternalInput").ap()
    Wv = nc.dram_tensor("Wv", (D, H), F32, kind="ExternalInput").ap()
    out = nc.dram_tensor("out", (T, H), F32, kind="ExternalOutput").ap()

    with tile.TileContext(nc) as tc:
        with tc.tile_pool(name="persist", bufs=1) as persist:
            ones_col = persist.tile([P, 1], BF16, name="ones_col")
            nc.vector.memset(ones_col, 1.0)
            QT = [persist.tile([P, T], BF16, name=f"qt{h}") for h in range(nH)]
            KT = [persist.tile([P, T], BF16, name=f"kt{h}") for h in range(nH)]
            V = [persist.tile([P, H], BF16, name=f"v{t}") for t in range(nT)]

            with tc.tile_pool(name="proj", bufs=1) as proj, \
                 tc.tile_pool(name="stage", bufs=3) as stage, \
                 tc.tile_pool(name="trpsum", bufs=3, space="PSUM") as trpsum, \
                 tc.tile_pool(name="ppsum", bufs=4, space="PSUM") as ppsum:
                from concourse.masks import make_identity

                ident = proj.tile([P, P], F32, name="ident")
                make_identity(nc, ident)

                xTa = proj.tile([P, nD, T], BF16, name="xTa")
                xT = [xTa[:, d] for d in range(nD)]
                Wqb = [proj.tile([P, H], BF16, name=f"wqb{d}") for d in range(nD)]
                Wkb = [proj.tile([P, H], BF16, name=f"wkb{d}") for d in range(nD)]
                Wvb = [proj.tile([P, H], BF16, name=f"wvb{d}") for d in range(nD)]

                # Weight loads: HWDGE f32 loads + engine cast to bf16.
                wcnt = 0
                for d in range(nD):
                    for Wsrc, Wb in ((Wq, Wqb), (Wk, Wkb), (Wv, Wvb)):
                        ws = stage.tile([P, H], F32, name=f"ws{wcnt}", tag="ws")
                        nc.sync.dma_start(ws, Wsrc[d * P:(d + 1) * P, :])
                        if wcnt % 2 == 0:
                            nc.vector.tensor_copy(Wb[d], ws)
                        else:
                            nc.scalar.copy(Wb[d], ws)
                        wcnt += 1

                # x: HWDGE f32 load, PE transpose (4 per PSUM bank), then one
                # PSUM->SBUF copy per bank with the bf16 cast folded in.
                for t in range(nT):
                    xs = stage.tile([P, D], F32, name=f"xs{t}", tag="xs")
                    nc.sync.dma_start(xs, x[t * P:(t + 1) * P, :])
                    for g in range(nD // 4):
                        tr = trpsum.tile([P, 4, P], F32, name=f"tr{t}_{g}",
                                         tag="tr")
                        for j in range(4):
                            d = 4 * g + j
                            nc.tensor.transpose(
                                tr[:, j],
                                xs[:, d * P:(d + 1) * P],
                                ident,
                            )
                        dst = xTa[:, 4 * g:4 * g + 4, t * P:(t + 1) * P]
                        if t % 2 == 0:
                            nc.vector.tensor_copy(dst, tr)
                        else:
                            nc.scalar.copy(dst, tr)

                # Projections. QT/KT: [h, t] = W*.T @ x.T ; V: [t, h] = x @ Wv.
                ncp = 0
                for Wb, OUT in ((Wqb, QT), (Wkb, KT)):
                    for h in range(nH):
                        for ts in range(nTS):
                            ps = ppsum.tile([P, STRIP], F32,
                                            name=f"ps{ncp}", tag="ps")
                            for d in range(nD):
                                nc.tensor.matmul(
                                    ps,
                                    Wb[d][:, h * P:(h + 1) * P],
                                    xT[d][:, ts * STRIP:(ts + 1) * STRIP],
                                    start=(d == 0), stop=(d == nD - 1),
                                )
                            dst = OUT[h][:, ts * STRIP:(ts + 1) * STRIP]
                            if ncp % 2 == 0:
                                nc.vector.tensor_copy(dst, ps)
                            else:
                                nc.scalar.copy(dst, ps)
                            ncp += 1
                for t in range(nT):
                    for hs in range(nHS):
                        ps = ppsum.tile([P, STRIP], F32, name=f"ps{ncp}", tag="ps")
                        for d in range(nD):
                            nc.tensor.matmul(
                                ps,
                                xT[d][:, t * P:(t + 1) * P],
                                Wvb[d][:, hs * STRIP:(hs + 1) * STRIP],
                                start=(d == 0), stop=(d == nD - 1),
                            )
                        if ncp % 2 == 0:
                            nc.vector.tensor_copy(
                                V[t][:, hs * STRIP:(hs + 1) * STRIP], ps)
                        else:
                            nc.scalar.copy(
                                V[t][:, hs * STRIP:(hs + 1) * STRIP], ps)
                        ncp += 1

            # Attention, strip by strip over tq.
            with tc.tile_pool(name="ptpool", bufs=2) as ptpool, \
                 tc.tile_pool(name="ostage", bufs=3) as ostage, \
                 tc.tile_pool(name="small", bufs=4) as small, \
                 tc.tile_pool(name="stpsum", bufs=2, space="PSUM") as stpsum, \
                 tc.tile_pool(name="opsum", bufs=2, space="PSUM") as opsum:
                for s in range(nTS):
                    q0 = s * STRIP
                    pts = []
                    for k in range((s + 1) * tps):  # tk tiles with any live tq
                        jq0 = max(0, k * P - q0)  # first unmasked col in strip
                        N = STRIP - jq0
                        st = stpsum.tile([P, STRIP], F32,
                                         name=f"st{s}_{k}", tag="st")
                        for h in range(nH):
                            nc.tensor.matmul(
                                st[:, :N],
                                KT[h][:, k * P:(k + 1) * P],
                                QT[h][:, q0 + jq0:q0 + STRIP],
                                start=(h == 0), stop=(h == nH - 1),
                            )
                        pt = ptpool.tile([P, STRIP], BF16,
                                         name=f"pt{s}_{k}", tag=f"pt{k}")
                        nc.scalar.activation(pt[:, jq0:STRIP], st[:, :N],
                                             EXP, scale=scale)
                        if k * P >= q0:
                            # Diagonal-crossing tile: zero where tk > tq.
                            # iota expr = j - p  (>=0 keeps exp, else 0).
                            nc.gpsimd.affine_select(
                                out=pt[:, jq0:STRIP], in_=pt[:, jq0:STRIP],
                                compare_op=mybir.AluOpType.is_ge,
                                fill=0.0, base=0, channel_multiplier=-1,
                                pattern=[[1, N]],
                            )
                        pts.append(pt)

                    for i in range(tps):
                        t = s * tps + i
                        ops = opsum.tile([P, H + 1], F32, name=f"o{t}", tag="o")
                        for k in range(t + 1):
                            lhsT = pts[k][:, i * P:(i + 1) * P]
                            first, last = (k == 0), (k == t)
                            for hs in range(nHS):
                                nc.tensor.matmul(
                                    ops[:, hs * STRIP:(hs + 1) * STRIP],
                                    lhsT,
                                    V[k][:, hs * STRIP:(hs + 1) * STRIP],
                                    start=first, stop=last,
                                )
                            nc.tensor.matmul(ops[:, H:H + 1], lhsT, ones_col,
                                             start=first, stop=last)
                        rinv = small.tile([P, 1], F32, name=f"rinv{t}",
                                          tag="rinv")
                        nc.vector.reciprocal(rinv, ops[:, H:H + 1])
                        osb = ostage.tile([P, H], F32, name=f"osb{t}", tag="osb")
                        nc.vector.tensor_scalar_mul(osb, ops[:, 0:H], rinv)
                        nc.sync.dma_start(out[t * P:(t + 1) * P, :], osb)

    nc.compile()
    return nc


def kernel(x, Wq, Wk, Wv):
    from concourse import bass_utils

    B, T, D = x.shape
    H = Wq.shape[1]
    nc = build_nc(T=T, D=D, H=H)
    in_maps = [
        {
            "x": np.ascontiguousarray(x[b], dtype=np.float32),
            "Wq": np.asarray(Wq, dtype=np.float32),
            "Wk": np.asarray(Wk, dtype=np.float32),
            "Wv": np.asarray(Wv, dtype=np.float32),
        }
        for b in range(B)
    ]
    res = bass_utils.run_bass_kernel_spmd(nc, in_maps, core_ids=list(range(B)))
    return np.stack([res.results[b]["out"] for b in range(B)], axis=0)


# revision 5
# speedup vs baseline: 1.4623x; 1.0225x over previous
"""Single-head causal self-attention for Trainium2, data-parallel over batch.

Problem: x[B=8, T=2048, D=1024], Wq/Wk/Wv[1024, 1024] (fp32).
  q/k/v = x @ W*, scores = (q @ k^T)/sqrt(H) causal-masked, out = softmax @ v.

Sharding: one batch element per NeuronCore (8 cores). Each core runs an
identical Bass/Tile program on its own x[b].

Per-core dataflow (all matmul compute in bf16, accumulation fp32):
  1. xT[d, t]  <- DMA-transpose of x (bf16) ; W* cast to bf16.
  2. QT[h, t] = Wq.T @ x.T ; KT likewise ; V[t, h] = x @ Wv.
  3. Scores are computed TRANSPOSED: ST[tk, tq] = K @ Q^T, so
     PT = exp(ST/sqrt(H)) (causal-masked via affine_select) is directly the
     stationary operand for O[tq, h] = PT.T @ V — no transposes of the
     softmax weights or the output are ever needed.
  4. Row-sums r[tq] accumulate in PSUM via an extra N=1 matmul against a
     ones column; O is normalized by 1/r during the PSUM->SBUF copy.
"""

import numpy as np

P = 128
STRIP = 512  # free-dim strip for N=512 matmuls (one fp32 PSUM bank)


def build_nc(T=2048, D=1024, H=1024):
    import concourse.bacc as bacc
    import concourse.mybir as mybir
    import concourse.tile as tile

    F32 = mybir.dt.float32
    BF16 = mybir.dt.bfloat16
    EXP = mybir.ActivationFunctionType.Exp

    nT, nD, nH = T // P, D // P, H // P
    nTS, nHS = T // STRIP, H // STRIP
    tps = STRIP // P  # t-tiles per strip
    scale = 1.0 / float(np.sqrt(H))

    nc = bacc.Bacc("TRN2", target_bir_lowering=False, debug=False)
    x = nc.dram_tensor("x", (T, D), F32, kind="ExternalInput").ap()
    Wq = nc.dram_tensor("Wq", (D, H), F32, kind="ExternalInput").ap()
    Wk = nc.dram_tensor("Wk", (D, H), F32, kind="ExternalInput").ap()
    Wv = nc.dram_tensor("Wv", (D, H), F32, kind="ExternalInput").ap()
    out = nc.dram_tensor("out", (T, H), F32, kind="ExternalOutput").ap()

    with tile.TileContext(nc) as tc:
        with tc.tile_pool(name="persist", bufs=1) as persist:
            ones_col = persist.tile([P, 1], BF16, name="ones_col")
            nc.vector.memset(ones_col, 1.0)
            QT = [persist.tile([P, T], BF16, name=f"qt{h}") for h in range(nH)]
            KT = [persist.tile([P, T], BF16, name=f"kt{h}") for h in range(nH)]
            V = [persist.tile([P, H], BF16, name=f"v{t}") for t in range(nT)]

            with tc.tile_pool(name="proj", bufs=1) as proj, \
                 tc.tile_pool(name="stage", bufs=3) as stage, \
                 tc.tile_pool(name="trpsum", bufs=3, space="PSUM") as trpsum, \
                 tc.tile_pool(name="ppsum", bufs=4, space="PSUM") as ppsum:
                from concourse.masks import make_identity

                ident = proj.tile([P, P], F32, name="ident")
                make_identity(nc, ident)

                xTa = proj.tile([P, nD, T], BF16, name="xTa")
                xT = [xTa[:, d] for d in range(nD)]
                Wqb = [proj.tile([P, H], BF16, name=f"wqb{d}") for d in range(nD)]
                Wkb = [proj.tile([P, H], BF16, name=f"wkb{d}") for d in range(nD)]
                Wvb = [proj.tile([P, H], BF16, name=f"wvb{d}") for d in range(nD)]

                # Weight loads: HWDGE f32 loads + engine cast to bf16.
                # Wv first — the V projection is the earliest PE consumer.
                wcnt = 0
                for Wsrc, Wb in ((Wv, Wvb), (Wq, Wqb), (Wk, Wkb)):
                    for d in range(nD):
                        ws = stage.tile([P, H], F32, name=f"ws{wcnt}", tag="ws")
                        nc.sync.dma_start(ws, Wsrc[d * P:(d + 1) * P, :])
                        if wcnt % 2 == 0:
                            nc.vector.tensor_copy(Wb[d], ws)
                        else:
                            nc.scalar.copy(Wb[d], ws)
                        wcnt += 1

                # Per x-tile: HWDGE f32 load, PE transpose (4 per PSUM bank),
                # one PSUM->SBUF copy per bank (bf16 cast folded in), then
                # immediately the V projection rows for this tile — PE gets
                # dense work as soon as the first x tile lands.
                ncp = 0
                for t in range(nT):
                    xs = stage.tile([P, D], F32, name=f"xs{t}", tag="xs")
                    nc.sync.dma_start(xs, x[t * P:(t + 1) * P, :])
                    for g in range(nD // 4):
                        tr = trpsum.tile([P, 4, P], F32, name=f"tr{t}_{g}",
                                         tag="tr")
                        for j in range(4):
                            d = 4 * g + j
                            nc.tensor.transpose(
                                tr[:, j],
                                xs[:, d * P:(d + 1) * P],
                                ident,
                            )
                        dst = xTa[:, 4 * g:4 * g + 4, t * P:(t + 1) * P]
                        if t % 2 == 0:
                            nc.vector.tensor_copy(dst, tr)
                        else:
                            nc.scalar.copy(dst, tr)
                    for hs in range(nHS):
                        ps = ppsum.tile([P, STRIP], F32, name=f"ps{ncp}",
                                        tag="ps")
                        for d in range(nD):
                            nc.tensor.matmul(
                                ps,
                                xT[d][:, t * P:(t + 1) * P],
                                Wvb[d][:, hs * STRIP:(hs + 1) * STRIP],
                                start=(d == 0), stop=(d == nD - 1),
                            )
                        if ncp % 2 == 0:
                            nc.vector.tensor_copy(
                                V[t][:, hs * STRIP:(hs + 1) * STRIP], ps)
                        else:
                            nc.scalar.copy(
                                V[t][:, hs * STRIP:(hs + 1) * STRIP], ps)
                        ncp += 1

                # QT/KT projections, t-strip OUTER so the attention phase can
                # begin as soon as strip 0 of both QT and KT is materialized.
                for ts in range(nTS):
                    for Wb, OUT in ((Wqb, QT), (Wkb, KT)):
                        for h in range(nH):
                            ps = ppsum.tile([P, STRIP], F32,
                                            name=f"ps{ncp}", tag="ps")
                            for d in range(nD):
                                nc.tensor.matmul(
                                    ps,
                                    Wb[d][:, h * P:(h + 1) * P],
                                    xT[d][:, ts * STRIP:(ts + 1) * STRIP],
                                    start=(d == 0), stop=(d == nD - 1),
                                )
                            dst = OUT[h][:, ts * STRIP:(ts + 1) * STRIP]
                            if ncp % 2 == 0:
                                nc.vector.tensor_copy(dst, ps)
                            else:
                                nc.scalar.copy(dst, ps)
                            ncp += 1

            # Attention, strip by strip over tq.
            with tc.tile_pool(name="ptpool", bufs=2) as ptpool, \
                 tc.tile_pool(name="ostage", bufs=3) as ostage, \
                 tc.tile_pool(name="small", bufs=4) as small, \
                 tc.tile_pool(name="stpsum", bufs=2, space="PSUM") as stpsum, \
                 tc.tile_pool(name="opsum", bufs=2, space="PSUM") as opsum:
                for s in range(nTS):
                    q0 = s * STRIP
                    pts = []
                    for k in range((s + 1) * tps):  # tk tiles with any live tq
                        jq0 = max(0, k * P - q0)  # first unmasked col in strip
                        N = STRIP - jq0
                        st = stpsum.tile([P, STRIP], F32,
                                         name=f"st{s}_{k}", tag="st")
                        for h in range(nH):
                            nc.tensor.matmul(
                                st[:, :N],
                                KT[h][:, k * P:(k + 1) * P],
                                QT[h][:, q0 + jq0:q0 + STRIP],
                                start=(h == 0), stop=(h == nH - 1),
                            )
                        pt = ptpool.tile([P, STRIP], BF16,
                                         name=f"pt{s}_{k}", tag=f"pt{k}")
                        nc.scalar.activation(pt[:, jq0:STRIP], st[:, :N],
                                             EXP, scale=scale)
                        if k * P >= q0:
                            # Diagonal-crossing tile: zero where tk > tq.
                            # iota expr = j - p  (>=0 keeps exp, else 0).
                            nc.gpsimd.affine_select(
                                out=pt[:, jq0:STRIP], in_=pt[:, jq0:STRIP],
                                compare_op=mybir.AluOpType.is_ge,
                                fill=0.0, base=0, channel_multiplier=-1,
                                pattern=[[1, N]],
                            )
                        pts.append(pt)

                    for i in range(tps):
                        t = s * tps + i
                        ops = opsum.tile([P, H + 1], F32, name=f"o{t}", tag="o")
                        for k in range(t + 1):
                            lhsT = pts[k][:, i * P:(i + 1) * P]
                            first, last = (k == 0), (k == t)
                            for hs in range(nHS):
                                nc.tensor.matmul(
                                    ops[:, hs * STRIP:(hs + 1) * STRIP],
                                    lhsT,
                                    V[k][:, hs * STRIP:(hs + 1) * STRIP],
                                    start=first, stop=last,
                                )
                            nc.tensor.matmul(ops[:, H:H + 1], lhsT, ones_col,
                                             start=first, stop=last)
                        rinv = small.tile([P, 1], F32, name=f"rinv{t}",
                                          tag="rinv")
                        nc.vector.reciprocal(rinv, ops[:, H:H + 1])
                        osb = ostage.tile([P, H], F32, name=f"osb{t}", tag="osb")
                        nc.vector.tensor_scalar_mul(osb, ops[:, 0:H], rinv)
                        nc.sync.dma_start(out[t * P:(t + 1) * P, :], osb)

    nc.compile()
    return nc


def kernel(x, Wq, Wk, Wv):
    from concourse import bass_utils

    B, T, D = x.shape
    H = Wq.shape[1]
    nc = build_nc(T=T, D=D, H=H)
    in_maps = [
        {
            "x": np.ascontiguousarray(x[b], dtype=np.float32),
            "Wq": np.asarray(Wq, dtype=np.float32),
            "Wk": np.asarray(Wk, dtype=np.float32),
            "Wv": np.asarray(Wv, dtype=np.float32),
        }
        for b in range(B)
    ]
    res = bass_utils.run_bass_kernel_spmd(nc, in_maps, core_ids=list(range(B)))
    return np.stack([res.results[b]["out"] for b in range(B)], axis=0)


# revision 9
# speedup vs baseline: 1.5965x; 1.0917x over previous
"""Single-head causal self-attention for Trainium2, data-parallel over batch.

Problem: x[B=8, T=2048, D=1024], Wq/Wk/Wv[1024, 1024] (fp32).
  q/k/v = x @ W*, scores = (q @ k^T)/sqrt(H) causal-masked, out = softmax @ v.

Sharding: one batch element per NeuronCore (8 cores). Each core runs an
identical Bass/Tile program on its own x[b].

Per-core dataflow (all matmul compute in bf16, accumulation fp32):
  1. xT[d, t]  <- DMA-transpose of x (bf16) ; W* cast to bf16.
  2. QT[h, t] = Wq.T @ x.T ; KT likewise ; V[t, h] = x @ Wv.
  3. Scores are computed TRANSPOSED: ST[tk, tq] = K @ Q^T, so
     PT = exp(ST/sqrt(H)) (causal-masked via affine_select) is directly the
     stationary operand for O[tq, h] = PT.T @ V — no transposes of the
     softmax weights or the output are ever needed.
  4. Row-sums r[tq] accumulate in PSUM via an extra N=1 matmul against a
     ones column; O is normalized by 1/r during the PSUM->SBUF copy.
"""

import numpy as np

P = 128
STRIP = 512  # free-dim strip for N=512 matmuls (one fp32 PSUM bank)


def build_nc(T=2048, D=1024, H=1024):
    import concourse.bacc as bacc
    import concourse.mybir as mybir
    import concourse.tile as tile

    F32 = mybir.dt.float32
    BF16 = mybir.dt.bfloat16
    EXP = mybir.ActivationFunctionType.Exp

    nT, nD, nH = T // P, D // P, H // P
    nTS, nHS = T // STRIP, H // STRIP
    tps = STRIP // P  # t-tiles per strip
    scale = 1.0 / float(np.sqrt(H))

    nc = bacc.Bacc("TRN2", target_bir_lowering=False, debug=False)
    x = nc.dram_tensor("x", (T, D), F32, kind="ExternalInput").ap()
    Wq = nc.dram_tensor("Wq", (D, H), F32, kind="ExternalInput").ap()
    Wk = nc.dram_tensor("Wk", (D, H), F32, kind="ExternalInput").ap()
    Wv = nc.dram_tensor("Wv", (D, H), F32, kind="ExternalInput").ap()
    out = nc.dram_tensor("out", (T, H), F32, kind="ExternalOutput").ap()

    with tile.TileContext(nc) as tc:
        with tc.tile_pool(name="persist", bufs=1) as persist:
            ones_col = persist.tile([P, 1], BF16, name="ones_col")
            nc.vector.memset(ones_col, 1.0)
            QT = [persist.tile([P, T], BF16, name=f"qt{h}") for h in range(nH)]
            KT = [persist.tile([P, T], BF16, name=f"kt{h}") for h in range(nH)]
            V = [persist.tile([P, H], BF16, name=f"v{t}") for t in range(nT)]

            with tc.tile_pool(name="proj", bufs=1) as proj, \
                 tc.tile_pool(name="stage", bufs=3) as stage, \
                 tc.tile_pool(name="trpsum", bufs=3, space="PSUM") as trpsum, \
                 tc.tile_pool(name="ppsum", bufs=4, space="PSUM") as ppsum:
                from concourse.masks import make_identity

                ident = proj.tile([P, P], F32, name="ident")
                make_identity(nc, ident)

                xTa = proj.tile([P, nD, T], BF16, name="xTa")
                xT = [xTa[:, d] for d in range(nD)]
                Wqb = [proj.tile([P, H], BF16, name=f"wqb{d}") for d in range(nD)]
                Wkb = [proj.tile([P, H], BF16, name=f"wkb{d}") for d in range(nD)]
                Wvb = [proj.tile([P, H], BF16, name=f"wvb{d}") for d in range(nD)]

                # Weight loads: HWDGE f32 loads + engine cast to bf16.
                # Emission order sets scheduler priority: x tile 0 and Wv
                # first (earliest PE consumers), Wq/Wk trail behind.
                wcnt = 0

                def load_w(Wsrc, Wb, d):
                    nonlocal wcnt
                    ws = stage.tile([P, H], F32, name=f"ws{wcnt}", tag="ws",
                                    bufs=4)
                    nc.sync.dma_start(ws, Wsrc[d * P:(d + 1) * P, :])
                    if wcnt % 2 == 0:
                        nc.vector.tensor_copy(Wb[d], ws)
                    else:
                        nc.scalar.copy(Wb[d], ws)
                    wcnt += 1

                for d in range(nD):
                    load_w(Wv, Wvb, d)

                # Per x-tile: HWDGE f32 load, PE transpose (4 per PSUM bank),
                # one PSUM->SBUF copy per bank (bf16 cast folded in), then
                # immediately the V projection rows for this tile — PE gets
                # dense work as soon as the first x tile lands.
                ncp = 0
                for t in range(nT):
                    xs = stage.tile([P, D], F32, name=f"xs{t}", tag="xs",
                                    bufs=3)
                    nc.sync.dma_start(xs, x[t * P:(t + 1) * P, :])
                    # Trickle the Wq/Wk loads in between the x tiles so they
                    # are resident well before the QT/KT phase needs them.
                    if t < nD:
                        load_w(Wq, Wqb, t)
                    elif t < 2 * nD:
                        load_w(Wk, Wkb, t - nD)
                    for g in range(nD // 4):
                        tr = trpsum.tile([P, 4, P], F32, name=f"tr{t}_{g}",
                                         tag="tr")
                        for j in range(4):
                            d = 4 * g + j
                            nc.tensor.transpose(
                                tr[:, j],
                                xs[:, d * P:(d + 1) * P],
                                ident,
                            )
                        dst = xTa[:, 4 * g:4 * g + 4, t * P:(t + 1) * P]
                        if t % 2 == 0:
                            nc.vector.tensor_copy(dst, tr)
                        else:
                            nc.scalar.copy(dst, tr)
                    for hs in range(nHS):
                        ps = ppsum.tile([P, STRIP], F32, name=f"ps{ncp}",
                                        tag="ps")
                        for d in range(nD):
                            nc.tensor.matmul(
                                ps,
                                xT[d][:, t * P:(t + 1) * P],
                                Wvb[d][:, hs * STRIP:(hs + 1) * STRIP],
                                start=(d == 0), stop=(d == nD - 1),
                            )
                        if ncp % 2 == 0:
                            nc.vector.tensor_copy(
                                V[t][:, hs * STRIP:(hs + 1) * STRIP], ps)
                        else:
                            nc.scalar.copy(
                                V[t][:, hs * STRIP:(hs + 1) * STRIP], ps)
                        ncp += 1

                # Any Wq/Wk tiles not covered by the trickle above (small T).
                for d in range(max(0, nT), nD):
                    load_w(Wq, Wqb, d)
                for d in range(max(0, nT - nD), nD):
                    load_w(Wk, Wkb, d)

                # QT/KT projections, t-strip OUTER so the attention phase can
                # begin as soon as strip 0 of both QT and KT is materialized.
                for ts in range(nTS):
                    for Wb, OUT in ((Wqb, QT), (Wkb, KT)):
                        for h in range(nH):
                            ps = ppsum.tile([P, STRIP], F32,
                                            name=f"ps{ncp}", tag="ps")
                            for d in range(nD):
                                nc.tensor.matmul(
                                    ps,
                                    Wb[d][:, h * P:(h + 1) * P],
                                    xT[d][:, ts * STRIP:(ts + 1) * STRIP],
                                    start=(d == 0), stop=(d == nD - 1),
                                )
                            dst = OUT[h][:, ts * STRIP:(ts + 1) * STRIP]
                            if ncp % 2 == 0:
                                nc.vector.tensor_copy(dst, ps)
                            else:
                                nc.scalar.copy(dst, ps)
                            ncp += 1

            # Attention, strip by strip over tq.
            with tc.tile_pool(name="ptpool", bufs=2) as ptpool, \
                 tc.tile_pool(name="ostage", bufs=3) as ostage, \
                 tc.tile_pool(name="small", bufs=4) as small, \
                 tc.tile_pool(name="stpsum", bufs=2, space="PSUM") as stpsum, \
                 tc.tile_pool(name="opsum", bufs=2, space="PSUM") as opsum:
                for s in range(nTS):
                    q0 = s * STRIP
                    pts = []
                    for k in range((s + 1) * tps):  # tk tiles with any live tq
                        jq0 = max(0, k * P - q0)  # first unmasked col in strip
                        N = STRIP - jq0
                        st = stpsum.tile([P, STRIP], F32,
                                         name=f"st{s}_{k}", tag="st")
                        for h in range(nH):
                            nc.tensor.matmul(
                                st[:, :N],
                                KT[h][:, k * P:(k + 1) * P],
                                QT[h][:, q0 + jq0:q0 + STRIP],
                                start=(h == 0), stop=(h == nH - 1),
                            )
                        pt = ptpool.tile([P, STRIP], BF16,
                                         name=f"pt{s}_{k}", tag=f"pt{k}")
                        nc.scalar.activation(pt[:, jq0:STRIP], st[:, :N],
                                             EXP, scale=scale)
                        if k * P >= q0:
                            # Diagonal-crossing tile: zero where tk > tq.
                            # iota expr = j - p  (>=0 keeps exp, else 0).
                            nc.gpsimd.affine_select(
                                out=pt[:, jq0:STRIP], in_=pt[:, jq0:STRIP],
                                compare_op=mybir.AluOpType.is_ge,
                                fill=0.0, base=0, channel_multiplier=-1,
                                pattern=[[1, N]],
                            )
                        pts.append(pt)

                    for i in range(tps):
                        t = s * tps + i
                        ops = opsum.tile([P, H + 1], F32, name=f"o{t}", tag="o")
                        for k in range(t + 1):
                            lhsT = pts[k][:, i * P:(i + 1) * P]
                            first, last = (k == 0), (k == t)
                            for hs in range(nHS):
                                nc.tensor.matmul(
                                    ops[:, hs * STRIP:(hs + 1) * STRIP],
                                    lhsT,
                                    V[k][:, hs * STRIP:(hs + 1) * STRIP],
                                    start=first, stop=last,
                                )
                            nc.tensor.matmul(ops[:, H:H + 1], lhsT, ones_col,
                                             start=first, stop=last)
                        rinv = small.tile([P, 1], F32, name=f"rinv{t}",
                                          tag="rinv")
                        nc.vector.reciprocal(rinv, ops[:, H:H + 1])
                        osb = ostage.tile([P, H], F32, name=f"osb{t}", tag="osb")
                        nc.vector.tensor_scalar_mul(osb, ops[:, 0:H], rinv)
                        nc.sync.dma_start(out[t * P:(t + 1) * P, :], osb)

    nc.compile()
    return nc


def kernel(x, Wq, Wk, Wv):
    from concourse import bass_utils

    B, T, D = x.shape
    H = Wq.shape[1]
    nc = build_nc(T=T, D=D, H=H)
    in_maps = [
        {
            "x": np.ascontiguousarray(x[b], dtype=np.float32),
            "Wq": np.asarray(Wq, dtype=np.float32),
            "Wk": np.asarray(Wk, dtype=np.float32),
            "Wv": np.asarray(Wv, dtype=np.float32),
        }
        for b in range(B)
    ]
    res = bass_utils.run_bass_kernel_spmd(nc, in_maps, core_ids=list(range(B)))
    return np.stack([res.results[b]["out"] for b in range(B)], axis=0)
